# revision 6
# baseline (speedup 1.0000x reference)
"""Criss-Cross Attention (CCA) Trainium2 Bass kernel.

Problem: n=8 images of (c=512, h=128, w=128); per-pixel projections
q,k (64ch) and v (512ch); row + column attention with joint softmax over
the 256 (w + h) logits per pixel (self pixel masked out of the column
branch); out = gamma * att + x.

Sharding: data-parallel over batch - one image per NeuronCore (8 cores).

Per-core pipeline (fp16 compute, fp32 PSUM):
  P1: stream x in 4-row blocks; project q,k channel-major (SBUF resident)
      and v PIXEL-major (lhsT = x chunks, rhs = Wv^T) -> vT_scr [H, W, C]
      in DRAM with contiguous 1KB+ descriptors.
  P2: two passes over e-matmuls. Sum pass: Z = sum(exp(e)) per pixel
      (fp32, no max subtraction). nb = -(ln Z - ln gamma) via exponent
      extraction. A-pass: a = exp(e + nb), the per-pixel nb applied as a
      per-partition ACT bias (no augmented channels / DRAM bounce);
      fp16 ring tiles batch-transposed (xbar DMA) into
      a_rowT [key, y, x_out] / a_colT [key, x, y_out].
  P3: NO v transposes: v row tiles [x_key, c] / col tiles [g, c] stream
      straight from vT_scr; apply matmuls use lhsT = v-tile slices so the
      output stays channel-major. Two 256-channel halves so the fp16
      col-sum accumulator is 8MB. Col pass writes acc; row pass computes
      out = row_att + acc + x and stores fp32.
"""

import sys

for _p in ("/opt/trn_rl_repo",):
    if _p not in sys.path:
        sys.path.insert(0, _p)

from contextlib import ExitStack

import numpy as np

from concourse import bacc
import concourse.bass as bass
import concourse.mybir as mybir
import concourse.tile as tile
from concourse.bass_utils import run_bass_kernel_spmd

F32 = mybir.dt.float32
F16 = mybir.dt.float16
AX = mybir.AxisListType
ALU = mybir.AluOpType
AF = mybir.ActivationFunctionType

N_CORES = 8
C, H, W = 512, 128, 128
CQK = 64
KC = 4  # input-channel chunks of 128
NEG_INF = -1e9


def build(n_cores: int = N_CORES):
    nc = bacc.Bacc("TRN2", debug=False, num_devices=n_cores)

    x_d = nc.dram_tensor("x", [C, H, W], F32, kind="ExternalInput")
    wq_d = nc.dram_tensor("Wq", [CQK, C], F32, kind="ExternalInput")
    bq_d = nc.dram_tensor("bq", [CQK], F32, kind="ExternalInput")
    wk_d = nc.dram_tensor("Wk", [CQK, C], F32, kind="ExternalInput")
    bk_d = nc.dram_tensor("bk", [CQK], F32, kind="ExternalInput")
    wv_d = nc.dram_tensor("Wv", [C, C], F32, kind="ExternalInput")
    bv_d = nc.dram_tensor("bv", [C], F32, kind="ExternalInput")
    g_d = nc.dram_tensor("gamma", [1], F32, kind="ExternalInput")
    out_d = nc.dram_tensor("out", [C, H, W], F32, kind="ExternalOutput")

    # v in pixel-major layout: vT_scr[y, x, c]
    vT_scr = nc.dram_tensor("vT_scr", [H, W, C], F16, kind="Internal")
    # fp16 copy of x for the P3 residual (identity-matmul accumulate)
    x16_scr = nc.dram_tensor("x16_scr", [C, H, W], F16, kind="Internal")

    with tile.TileContext(nc) as tc, ExitStack() as ctx:
        cst2 = ctx.enter_context(tc.tile_pool(name="cst2", bufs=1))
        ident32 = cst2.tile([128, 128], F32)
        from concourse.masks import make_identity

        make_identity(nc, ident32)
        ident16 = cst2.tile([128, 128], F16)
        nc.vector.tensor_copy(ident16, ident32)

        # ---- persistent attention maps (allocated at stack bottom) -------
        a_rowT = ctx.enter_context(tc.tile_pool(name="a_rowT", bufs=1))
        a_colT = ctx.enter_context(tc.tile_pool(name="a_colT", bufs=1))
        a_rowT_t = a_rowT.tile([128, H, 128], F16)  # (key x, y, x_out)
        a_colT_t = a_colT.tile([128, W, 128], F16)  # (key g, x, y_out)

        # ==================================================================
        # P1 + P2 transients in a nested scope (freed before P3)
        # ==================================================================
        with ExitStack() as p12:
            const = p12.enter_context(tc.tile_pool(name="const", bufs=1))
            stats = p12.enter_context(tc.tile_pool(name="stats", bufs=1))

            diag_neg4 = const.tile([128, 4, 128], F32)
            nc.gpsimd.memset(diag_neg4, 0.0)
            nc.gpsimd.affine_select(
                out=diag_neg4,
                in_=diag_neg4,
                compare_op=ALU.not_equal,
                fill=NEG_INF,
                base=0,
                pattern=[[0, 4], [-1, 128]],
                channel_multiplier=1,
            )

            bq_sb = const.tile([CQK, 1], F32)
            nc.sync.dma_start(
                out=bq_sb, in_=bq_d[:].rearrange("(a b) -> a b", b=1)
            )
            bk_sb = const.tile([CQK, 1], F32)
            nc.sync.dma_start(
                out=bk_sb, in_=bk_d[:].rearrange("(a b) -> a b", b=1)
            )
            g_ap = g_d[:]
            g_bcast = bass.AP(
                tensor=g_ap.tensor, offset=g_ap.offset, ap=[[0, 128], [1, 1]]
            )
            g_sb = const.tile([128, 1], F32)
            nc.gpsimd.dma_start(out=g_sb, in_=g_bcast)
            lng = stats.tile([128, 1], F32)
            nc.scalar.activation(lng, g_sb, AF.Ln)

            # bv broadcast to [128, 512] via rank-1 matmul (ones^T @ bv)
            ones_row = const.tile([1, 128], F16)
            nc.gpsimd.memset(ones_row, 1.0)
            bv_row32 = const.tile([1, C], F32)
            nc.sync.dma_start(
                out=bv_row32, in_=bv_d[:].rearrange("(a b) -> a b", a=1)
            )
            bv_row = const.tile([1, C], F16)
            nc.vector.tensor_copy(bv_row, bv_row32)

            # transposed projection weights (fp16): wqkT [128, KC, 128]
            # (cols 0:64 = Wq^T chunk, 64:128 = Wk^T chunk);
            # wvT [128, KC, 512] = Wv^T chunks
            wqkT = const.tile([128, KC, 128], F16)
            wvT = const.tile([128, KC, C], F16)
            with tc.tile_pool(name="wprep", bufs=2) as wprep, tc.tile_pool(
                name="wps", bufs=2, space="PSUM"
            ) as wps:
                for kc in range(KC):
                    for w_d, col0 in ((wq_d, 0), (wk_d, CQK)):
                        raw = wprep.tile([CQK, 128], F32, tag="rawqk")
                        nc.sync.dma_start(
                            out=raw, in_=w_d[:, kc * 128 : (kc + 1) * 128]
                        )
                        tps = wps.tile([128, CQK], F32, tag="tqk")
                        nc.tensor.transpose(tps, raw, ident32[:CQK, :CQK])
                        nc.vector.tensor_copy(
                            wqkT[:, kc, col0 : col0 + CQK], tps
                        )
                    for oc in range(4):
                        rawv = wprep.tile([128, 128], F32, tag="rawv")
                        nc.sync.dma_start(
                            out=rawv,
                            in_=wv_d[
                                oc * 128 : (oc + 1) * 128,
                                kc * 128 : (kc + 1) * 128,
                            ],
                        )
                        tps2 = wps.tile([128, 128], F32, tag="tv")
                        nc.tensor.transpose(tps2, rawv, ident32)
                        nc.vector.tensor_copy(
                            wvT[:, kc, oc * 128 : (oc + 1) * 128], tps2
                        )

            qk = p12.enter_context(tc.tile_pool(name="qk", bufs=1))
            q_sb = qk.tile([CQK, H, W], F16)  # (c, y, x)
            k_sb = qk.tile([CQK, H, W], F16)

            s1 = stats.tile([128, H], F32)  # [x, y] row-branch exp sums
            s2 = stats.tile([128, W], F32)  # [y, x] col-branch exp sums
            nb_yx = stats.tile([128, W], F32)  # [y, x] = -(lnZ - ln g)
            nb_xy = stats.tile([128, H], F32)  # [x, y]

            # ---------------- P1: projections -----------------------------
            with tc.tile_pool(name="xin", bufs=3) as xin, tc.tile_pool(
                name="x16", bufs=3
            ) as x16p, tc.tile_pool(name="v16", bufs=3) as v16p, tc.tile_pool(
                name="p1ps", bufs=1, space="PSUM"
            ) as p1ps:
                for b in range(H // 4):
                    y0 = 4 * b
                    xt = xin.tile([128, KC, 512], F32, tag="xt")
                    for kc in range(KC):
                        nc.sync.dma_start(
                            out=xt[:, kc, :],
                            in_=x_d[
                                kc * 128 : (kc + 1) * 128, y0 : y0 + 4, :
                            ].rearrange("c r w -> c (r w)"),
                        )
                    x16 = x16p.tile([128, KC, 512], F16, tag="x16")
                    nc.vector.tensor_copy(
                        x16.rearrange("c k w -> c (k w)"),
                        xt.rearrange("c k w -> c (k w)"),
                    )
                    nc.scalar.dma_start(
                        out=x16_scr[:, y0 : y0 + 4, :].rearrange(
                            "(k c) y x -> c k (y x)", k=KC
                        ),
                        in_=x16,
                    )

                    # q,k channel-major: accumulate over kc
                    qk_ps = p1ps.tile([128, 512], F32, tag="qkps", bufs=2)
                    for kc in range(KC):
                        nc.tensor.matmul(
                            qk_ps,
                            wqkT[:, kc, :],
                            x16[:, kc, :],
                            start=(kc == 0),
                            stop=(kc == KC - 1),
                        )
                    nc.vector.tensor_scalar_add(
                        q_sb[:, y0 : y0 + 4, :].rearrange("c r w -> c (r w)"),
                        qk_ps[0:CQK, :],
                        bq_sb,
                    )
                    nc.vector.tensor_scalar_add(
                        k_sb[:, y0 : y0 + 4, :].rearrange("c r w -> c (r w)"),
                        qk_ps[CQK:128, :],
                        bk_sb,
                    )

                    # v pixel-major: per row y, out[x, c512] with
                    # lhsT = x16 chunk [cin, x], rhs = wvT [cin, c512]
                    vt16 = v16p.tile([128, 4, C], F16, tag="v16")
                    for j in range(4):
                        v_ps = p1ps.tile([128, C], F32, tag="vps", bufs=4)
                        for kc in range(KC):
                            nc.tensor.matmul(
                                v_ps,
                                x16[:, kc, j * 128 : (j + 1) * 128],
                                wvT[:, kc, :],
                                start=(kc == 0),
                                stop=False,
                            )
                        nc.tensor.matmul(
                            v_ps, ones_row, bv_row, start=False, stop=True
                        )
                        if j % 2 == 0:
                            nc.vector.tensor_copy(vt16[:, j, :], v_ps)
                        else:
                            nc.scalar.copy(vt16[:, j, :], v_ps)
                    nc.sync.dma_start(
                        out=vT_scr[y0 : y0 + 4, :, :].rearrange(
                            "y x c -> x y c"
                        ),
                        in_=vt16,
                    )

            # ---------------- P2: softmax ----------------------------------
            trash = p12.enter_context(tc.tile_pool(name="trash", bufs=4))
            emsk = p12.enter_context(tc.tile_pool(name="emsk", bufs=4))
            ring = p12.enter_context(tc.tile_pool(name="ring", bufs=2))

            with tc.tile_pool(name="p2ps", bufs=1, space="PSUM") as p2ps:
                # ---- sum pass (fp32 exp; no max subtraction) -------------
                for y0 in range(0, H, 4):
                    e_ps = p2ps.tile([128, 4, 128], F32, tag="e_ps", bufs=4)
                    for j in range(4):
                        nc.tensor.matmul(
                            e_ps[:, j, :],
                            q_sb[:, y0 + j, :],
                            k_sb[:, y0 + j, :],
                            start=True,
                            stop=True,
                        )
                    tr = trash.tile([128, 4, 128], F32, tag="trash")
                    nc.scalar.activation(
                        tr.rearrange("p a b -> p (a b)"),
                        e_ps.rearrange("p a b -> p (a b)"),
                        AF.Exp,
                    )
                    nc.vector.reduce_sum(s1[:, y0 : y0 + 4], tr, axis=AX.X)
                for x0 in range(0, W, 4):
                    e_ps = p2ps.tile([128, 4, 128], F32, tag="e_ps", bufs=4)
                    for j in range(4):
                        nc.tensor.matmul(
                            e_ps[:, j, :],
                            q_sb[:, :, x0 + j],
                            k_sb[:, :, x0 + j],
                            start=True,
                            stop=True,
                        )
                    em = emsk.tile([128, 4, 128], F32, tag="emsk")
                    nc.vector.tensor_tensor(
                        em.rearrange("p a b -> p (a b)"),
                        e_ps.rearrange("p a b -> p (a b)"),
                        diag_neg4.rearrange("p a b -> p (a b)"),
                        ALU.add,
                    )
                    tr = trash.tile([128, 4, 128], F32, tag="trash")
                    nc.scalar.activation(
                        tr.rearrange("p a b -> p (a b)"),
                        em.rearrange("p a b -> p (a b)"),
                        AF.Exp,
                    )
                    nc.vector.reduce_sum(s2[:, x0 : x0 + 4], tr, axis=AX.X)

                # ---- nb[y,x] = -(ln(Z) - ln(gamma)); ln via exponent
                # extraction so any fp32 Z is in the ACT Ln table range ----
                zt_ps = p2ps.tile([128, 128], F32, tag="zt", bufs=1)
                nc.tensor.transpose(zt_ps, s1, ident32)
                z_yx = stats.tile([128, W], F32)
                nc.vector.tensor_tensor(z_yx, zt_ps, s2, ALU.add)
                z_i = z_yx[...].bitcast(mybir.dt.int32)
                e_i32 = stats.tile([128, W], mybir.dt.int32)
                nc.vector.tensor_scalar(
                    out=e_i32,
                    in0=z_i,
                    scalar1=23,
                    scalar2=None,
                    op0=ALU.logical_shift_right,
                )
                ef = stats.tile([128, W], F32)
                nc.vector.tensor_scalar(
                    out=ef,
                    in0=e_i32,
                    scalar1=127,
                    scalar2=None,
                    op0=ALU.subtract,
                )
                mant = stats.tile([128, W], F32)
                nc.vector.tensor_scalar(
                    out=mant[...].bitcast(mybir.dt.int32),
                    in0=z_i,
                    scalar1=0x007FFFFF,
                    scalar2=0x3F800000,
                    op0=ALU.bitwise_and,
                    op1=ALU.bitwise_or,
                )
                lnm = stats.tile([128, W], F32)
                nc.scalar.activation(lnm, mant, AF.Ln)
                lnz = stats.tile([128, W], F32)
                nc.vector.scalar_tensor_tensor(
                    out=lnz,
                    in0=ef,
                    scalar=float(np.log(2.0)),
                    in1=lnm,
                    op0=ALU.mult,
                    op1=ALU.add,
                )
                nc.vector.tensor_scalar(
                    out=nb_yx,
                    in0=lnz,
                    scalar1=lng,
                    scalar2=-1.0,
                    op0=ALU.subtract,
                    op1=ALU.mult,
                )
                # transpose nb to [x, y] for the row a-pass bias
                nbt_ps = p2ps.tile([128, 128], F32, tag="zt", bufs=1)
                nc.tensor.transpose(nbt_ps, nb_yx, ident32)
                nc.vector.tensor_copy(nb_xy, nbt_ps)

                # ---- a passes: a = exp(e + nb) via per-partition ACT bias,
                # through ring buffers + batched xbar transpose ------------
                for ycb in range(H // 16):
                    rt = ring.tile([128, 16, 128], F16, tag="ring")
                    for j4 in range(4):
                        y0 = ycb * 16 + j4 * 4
                        e_ps = p2ps.tile(
                            [128, 4, 128], F32, tag="e_ps", bufs=4
                        )
                        for j in range(4):
                            nc.tensor.matmul(
                                e_ps[:, j, :],
                                q_sb[:, y0 + j, :],
                                k_sb[:, y0 + j, :],
                                start=True,
                                stop=True,
                            )
                        for j in range(4):
                            nc.scalar.activation(
                                rt[:, j4 * 4 + j, :],
                                e_ps[:, j, :],
                                AF.Exp,
                                bias=nb_xy[:, y0 + j : y0 + j + 1],
                            )
                    nc.sync.dma_start(
                        out=a_rowT_t[:, ycb * 16 : (ycb + 1) * 16, :],
                        in_=rt.rearrange("p a b -> p (a b)"),
                        transpose=True,
                    )
                for xcb in range(W // 16):
                    rt = ring.tile([128, 16, 128], F16, tag="ring")
                    for j4 in range(4):
                        x0 = xcb * 16 + j4 * 4
                        e_ps = p2ps.tile(
                            [128, 4, 128], F32, tag="e_ps", bufs=4
                        )
                        for j in range(4):
                            nc.tensor.matmul(
                                e_ps[:, j, :],
                                q_sb[:, :, x0 + j],
                                k_sb[:, :, x0 + j],
                                start=True,
                                stop=True,
                            )
                        for j in range(4):
                            nc.scalar.activation(
                                rt[:, j4 * 4 + j, :],
                                e_ps[:, j, :],
                                AF.Exp,
                                bias=nb_yx[:, x0 + j : x0 + j + 1],
                            )
                    nc.sync.dma_start(
                        out=a_colT_t[:, xcb * 16 : (xcb + 1) * 16, :],
                        in_=rt.rearrange("p a b -> p (a b)"),
                        transpose=True,
                    )
                # self pixel: zero the (g == y_out) diagonal across all x
                nc.gpsimd.affine_select(
                    out=a_colT_t,
                    in_=a_colT_t,
                    compare_op=ALU.not_equal,
                    fill=0.0,
                    base=0,
                    pattern=[[0, W], [-1, 128]],
                    channel_multiplier=1,
                )

        # ==================================================================
        # P3: attention application, channel-major output, no transposes.
        # Two 256-channel halves; col pass writes acc, row pass adds
        # acc + x and stores.
        # ==================================================================
        with ExitStack() as p3:
            accp = p3.enter_context(tc.tile_pool(name="accp", bufs=1))
            vrow = p3.enter_context(tc.tile_pool(name="vrow", bufs=3))
            vcol = p3.enter_context(tc.tile_pool(name="vcol", bufs=3))
            xres = p3.enter_context(tc.tile_pool(name="xres", bufs=3))
            outp = p3.enter_context(tc.tile_pool(name="outp", bufs=3))
            tmpp = p3.enter_context(tc.tile_pool(name="tmpp", bufs=3))

            for ch in range(2):  # channel half: c in [ch*256, (ch+1)*256)
                c0 = ch * 256
                acc = accp.tile([128, 2, H, W], F16, tag="acc")

                # ---- col pass: acc[c, o, y, x] = col attention ----------
                with tc.tile_pool(name=f"colps{ch}", bufs=1, space="PSUM") as cps:
                    for xg in range(W // 4):
                        x0 = 4 * xg
                        vc = vcol.tile([128, 4, 256], F16, tag="vc")
                        nc.sync.dma_start(
                            out=vc,
                            in_=vT_scr[:, x0 : x0 + 4, c0 : c0 + 256],
                        )
                        for xi in range(4):
                            x = x0 + xi
                            cc_ps = cps.tile(
                                [128, 2, 128], F32, tag="ccps", bufs=4
                            )
                            for o in range(2):
                                nc.tensor.matmul(
                                    cc_ps[:, o, :],
                                    vc[:, xi, o * 128 : (o + 1) * 128],
                                    a_colT_t[:, x, :],
                                    start=True,
                                    stop=True,
                                )
                            if xi % 2 == 0:
                                nc.vector.tensor_copy(acc[:, :, :, x], cc_ps)
                            else:
                                nc.scalar.copy(acc[:, :, :, x], cc_ps)

                # ---- row pass: out = row attention + acc + x ------------
                with tc.tile_pool(name=f"rowps{ch}", bufs=1, space="PSUM") as rps:
                    for yg in range(H // 4):
                        y0 = 4 * yg
                        vr = vrow.tile([128, 4, 256], F16, tag="vr")
                        nc.scalar.dma_start(
                            out=vr,
                            in_=vT_scr[y0 : y0 + 4, :, c0 : c0 + 256].rearrange(
                                "y x c -> x y c"
                            ),
                        )
                        for o in range(2):
                            co = c0 + o * 128
                            xr = xres.tile([128, 4, 128], F16, tag="xr")
                            nc.scalar.dma_start(
                                out=xr.rearrange("c r w -> c (r w)"),
                                in_=x16_scr[
                                    co : co + 128, y0 : y0 + 4, :
                                ].rearrange("c r w -> c (r w)"),
                            )
                            or_ps = rps.tile(
                                [128, 4, 128], F32, tag="orps", bufs=4
                            )
                            for j in range(4):
                                nc.tensor.matmul(
                                    or_ps[:, j, :],
                                    vr[:, j, o * 128 : (o + 1) * 128],
                                    a_rowT_t[:, y0 + j, :],
                                    start=(j == 0),
                                    stop=False,
                                )
                            nc.tensor.matmul(
                                or_ps.rearrange("c r w -> c (r w)"),
                                ident16,
                                acc[:, o, y0 : y0 + 4, :].rearrange(
                                    "c r w -> c (r w)"
                                ),
                                start=False,
                                stop=False,
                            )
                            nc.tensor.matmul(
                                or_ps.rearrange("c r w -> c (r w)"),
                                ident16,
                                xr.rearrange("c r w -> c (r w)"),
                                start=False,
                                stop=True,
                            )
                            ot = outp.tile([128, 4, 128], F32, tag="ot")
                            if o % 2 == 0:
                                nc.vector.tensor_copy(
                                    ot.rearrange("c r w -> c (r w)"),
                                    or_ps.rearrange("c r w -> c (r w)"),
                                )
                            else:
                                nc.scalar.copy(
                                    ot.rearrange("c r w -> c (r w)"),
                                    or_ps.rearrange("c r w -> c (r w)"),
                                )
                            nc.sync.dma_start(
                                out=out_d[
                                    co : co + 128, y0 : y0 + 4, :
                                ].rearrange("c r w -> c (r w)"),
                                in_=ot.rearrange("p a b -> p (a b)"),
                            )

    nc.finalize()
    return nc


_NC_CACHE = {}


def _get_nc():
    if "nc" not in _NC_CACHE:
        _NC_CACHE["nc"] = build()
    return _NC_CACHE["nc"]


def kernel(**inputs) -> np.ndarray:
    x = np.ascontiguousarray(np.asarray(inputs["x"], dtype=np.float32))
    n = x.shape[0]
    assert x.shape == (n, C, H, W)
    shared = {
        name: np.ascontiguousarray(np.asarray(inputs[name], dtype=np.float32))
        for name in ("Wq", "bq", "Wk", "bk", "Wv", "bv", "gamma")
    }
    nc = _get_nc()
    in_maps = [{"x": x[i], **shared} for i in range(n)]
    res = run_bass_kernel_spmd(nc, in_maps, core_ids=list(range(n)))
    return np.stack([res.results[i]["out"] for i in range(n)], axis=0)


if __name__ == "__main__":
    rng = np.random.default_rng(0)
    demo = {
        "x": rng.standard_normal((N_CORES, C, H, W), dtype=np.float32),
        "Wq": rng.standard_normal((CQK, C), dtype=np.float32) / np.sqrt(C),
        "bq": np.zeros(CQK, np.float32),
        "Wk": rng.standard_normal((CQK, C), dtype=np.float32) / np.sqrt(C),
        "bk": np.zeros(CQK, np.float32),
        "Wv": rng.standard_normal((C, C), dtype=np.float32) / np.sqrt(C),
        "bv": np.zeros(C, np.float32),
        "gamma": np.ones(1, np.float32),
    }
    out = kernel(**demo)
    print("out", out.shape, out.dtype, np.abs(out).mean())


# revision 11
# speedup vs baseline: 1.1312x; 1.1312x over previous
"""Criss-Cross Attention (CCA) Trainium2 Bass kernel.

Problem: n=8 images of (c=512, h=128, w=128); per-pixel projections
q,k (64ch) and v (512ch); row + column attention with joint softmax over
the 256 (w + h) logits per pixel (self pixel masked out of the column
branch); out = gamma * att + x.

Sharding: data-parallel over batch - one image per NeuronCore (8 cores).

Per-core pipeline (fp16 compute, fp32 PSUM):
  P1: stream x in 4-row blocks; project q,k channel-major (SBUF resident)
      and v PIXEL-major (lhsT = x chunks, rhs = Wv^T) -> vT_scr [H, W, C]
      in DRAM with contiguous 1KB+ descriptors.
  P2: two passes over e-matmuls. Sum pass: Z = sum(exp(e)) per pixel
      (fp32, no max subtraction). nb = -(ln Z - ln gamma) via exponent
      extraction. A-pass: a = exp(e + nb), the per-pixel nb applied as a
      per-partition ACT bias (no augmented channels / DRAM bounce);
      fp16 ring tiles batch-transposed (xbar DMA) into
      a_rowT [key, y, x_out] / a_colT [key, x, y_out].
  P3: NO v transposes: v row tiles [x_key, c] / col tiles [g, c] stream
      straight from vT_scr; apply matmuls use lhsT = v-tile slices so the
      output stays channel-major. Two 256-channel halves so the fp16
      col-sum accumulator is 8MB. Col pass writes acc; row pass computes
      out = row_att + acc + x and stores fp32.
"""

import sys

for _p in ("/opt/trn_rl_repo",):
    if _p not in sys.path:
        sys.path.insert(0, _p)

from contextlib import ExitStack

import numpy as np

from concourse import bacc
import concourse.bass as bass
import concourse.mybir as mybir
import concourse.tile as tile
from concourse.bass_utils import run_bass_kernel_spmd

F32 = mybir.dt.float32
F16 = mybir.dt.float16
AX = mybir.AxisListType
ALU = mybir.AluOpType
AF = mybir.ActivationFunctionType

N_CORES = 8
C, H, W = 512, 128, 128
CQK = 64
KC = 4  # input-channel chunks of 128
NEG_INF = -1e9


def build(n_cores: int = N_CORES):
    nc = bacc.Bacc("TRN2", debug=False, num_devices=n_cores)

    x_d = nc.dram_tensor("x", [C, H, W], F32, kind="ExternalInput")
    wq_d = nc.dram_tensor("Wq", [CQK, C], F32, kind="ExternalInput")
    bq_d = nc.dram_tensor("bq", [CQK], F32, kind="ExternalInput")
    wk_d = nc.dram_tensor("Wk", [CQK, C], F32, kind="ExternalInput")
    bk_d = nc.dram_tensor("bk", [CQK], F32, kind="ExternalInput")
    wv_d = nc.dram_tensor("Wv", [C, C], F32, kind="ExternalInput")
    bv_d = nc.dram_tensor("bv", [C], F32, kind="ExternalInput")
    g_d = nc.dram_tensor("gamma", [1], F32, kind="ExternalInput")
    out_d = nc.dram_tensor("out", [C, H, W], F32, kind="ExternalOutput")

    # v in pixel-major layout: vT_scr[y, x, c]
    vT_scr = nc.dram_tensor("vT_scr", [H, W, C], F16, kind="Internal")
    # fp16 copy of x for the P3 residual (identity-matmul accumulate)
    x16_scr = nc.dram_tensor("x16_scr", [C, H, W], F16, kind="Internal")

    with tile.TileContext(nc) as tc, ExitStack() as ctx:
        cst2 = ctx.enter_context(tc.tile_pool(name="cst2", bufs=1))
        ident32 = cst2.tile([128, 128], F32)
        from concourse.masks import make_identity

        make_identity(nc, ident32)
        ident16 = cst2.tile([128, 128], F16)
        nc.vector.tensor_copy(ident16, ident32)

        # ---- persistent attention maps (allocated at stack bottom) -------
        a_rowT = ctx.enter_context(tc.tile_pool(name="a_rowT", bufs=1))
        a_colT = ctx.enter_context(tc.tile_pool(name="a_colT", bufs=1))
        a_rowT_t = a_rowT.tile([128, H, 128], F16)  # (key x, y, x_out)
        a_colT_t = a_colT.tile([128, W, 128], F16)  # (key g, x, y_out)

        # ==================================================================
        # P1 + P2 transients in a nested scope (freed before P3)
        # ==================================================================
        with ExitStack() as p12:
            const = p12.enter_context(tc.tile_pool(name="const", bufs=1))
            stats = p12.enter_context(tc.tile_pool(name="stats", bufs=1))

            diag_neg4 = const.tile([128, 4, 128], F32)
            nc.gpsimd.memset(diag_neg4, 0.0)
            nc.gpsimd.affine_select(
                out=diag_neg4,
                in_=diag_neg4,
                compare_op=ALU.not_equal,
                fill=NEG_INF,
                base=0,
                pattern=[[0, 4], [-1, 128]],
                channel_multiplier=1,
            )

            bq_sb = const.tile([CQK, 1], F32)
            nc.sync.dma_start(
                out=bq_sb, in_=bq_d[:].rearrange("(a b) -> a b", b=1)
            )
            bk_sb = const.tile([CQK, 1], F32)
            nc.sync.dma_start(
                out=bk_sb, in_=bk_d[:].rearrange("(a b) -> a b", b=1)
            )
            g_ap = g_d[:]
            g_bcast = bass.AP(
                tensor=g_ap.tensor, offset=g_ap.offset, ap=[[0, 128], [1, 1]]
            )
            g_sb = const.tile([128, 1], F32)
            nc.gpsimd.dma_start(out=g_sb, in_=g_bcast)
            lng = stats.tile([128, 1], F32)
            nc.scalar.activation(lng, g_sb, AF.Ln)

            # bv broadcast to [128, 512] via rank-1 matmul (ones^T @ bv)
            ones_row = const.tile([1, 128], F16)
            nc.gpsimd.memset(ones_row, 1.0)
            bv_row32 = const.tile([1, C], F32)
            nc.sync.dma_start(
                out=bv_row32, in_=bv_d[:].rearrange("(a b) -> a b", a=1)
            )
            bv_row = const.tile([1, C], F16)
            nc.vector.tensor_copy(bv_row, bv_row32)
            bv_full = const.tile([128, C], F32)
            with tc.tile_pool(name="bvps", bufs=1, space="PSUM") as bvps:
                bv_ps = bvps.tile([128, C], F32)
                nc.tensor.matmul(
                    bv_ps, ones_row, bv_row, start=True, stop=True
                )
                nc.vector.tensor_copy(bv_full, bv_ps)

            # transposed projection weights (fp16): wqkT [128, KC, 128]
            # (cols 0:64 = Wq^T chunk, 64:128 = Wk^T chunk);
            # wvT [128, KC, 512] = Wv^T chunks
            wqkT = const.tile([128, KC, 128], F16)
            wvT = const.tile([128, KC, C], F16)
            with tc.tile_pool(name="wprep", bufs=2) as wprep, tc.tile_pool(
                name="wps", bufs=2, space="PSUM"
            ) as wps:
                for kc in range(KC):
                    for w_d, col0 in ((wq_d, 0), (wk_d, CQK)):
                        raw = wprep.tile([CQK, 128], F32, tag="rawqk")
                        nc.sync.dma_start(
                            out=raw, in_=w_d[:, kc * 128 : (kc + 1) * 128]
                        )
                        tps = wps.tile([128, CQK], F32, tag="tqk")
                        nc.tensor.transpose(tps, raw, ident32[:CQK, :CQK])
                        nc.vector.tensor_copy(
                            wqkT[:, kc, col0 : col0 + CQK], tps
                        )
                    for oc in range(4):
                        rawv = wprep.tile([128, 128], F32, tag="rawv")
                        nc.sync.dma_start(
                            out=rawv,
                            in_=wv_d[
                                oc * 128 : (oc + 1) * 128,
                                kc * 128 : (kc + 1) * 128,
                            ],
                        )
                        tps2 = wps.tile([128, 128], F32, tag="tv")
                        nc.tensor.transpose(tps2, rawv, ident32)
                        nc.vector.tensor_copy(
                            wvT[:, kc, oc * 128 : (oc + 1) * 128], tps2
                        )

            qk = p12.enter_context(tc.tile_pool(name="qk", bufs=1))
            q_sb = qk.tile([CQK, H, W], F16)  # (c, y, x)
            k_sb = qk.tile([CQK, H, W], F16)

            s1 = stats.tile([128, H], F32)  # [x, y] row-branch exp sums
            s2 = stats.tile([128, W], F32)  # [y, x] col-branch exp sums
            nb_yx = stats.tile([128, W], F32)  # [y, x] = -(lnZ - ln g)
            nb_xy = stats.tile([128, H], F32)  # [x, y]

            # ---------------- P1: projections -----------------------------
            with tc.tile_pool(name="xin", bufs=3) as xin, tc.tile_pool(
                name="x16", bufs=3
            ) as x16p, tc.tile_pool(name="v16", bufs=3) as v16p, tc.tile_pool(
                name="p1ps", bufs=1, space="PSUM"
            ) as p1ps:
                for b in range(H // 4):
                    y0 = 4 * b
                    xt = xin.tile([128, KC, 512], F32, tag="xt")
                    for kc in range(KC):
                        nc.sync.dma_start(
                            out=xt[:, kc, :],
                            in_=x_d[
                                kc * 128 : (kc + 1) * 128, y0 : y0 + 4, :
                            ].rearrange("c r w -> c (r w)"),
                        )
                    x16 = x16p.tile([128, KC, 512], F16, tag="x16")
                    nc.vector.tensor_copy(
                        x16.rearrange("c k w -> c (k w)"),
                        xt.rearrange("c k w -> c (k w)"),
                    )
                    nc.gpsimd.dma_start(
                        out=x16_scr[:, y0 : y0 + 4, :].rearrange(
                            "(k c) y x -> c k (y x)", k=KC
                        ),
                        in_=x16,
                    )

                    # q,k channel-major: accumulate over kc
                    qk_ps = p1ps.tile([128, 512], F32, tag="qkps", bufs=2)
                    for kc in range(KC):
                        nc.tensor.matmul(
                            qk_ps,
                            wqkT[:, kc, :],
                            x16[:, kc, :],
                            start=(kc == 0),
                            stop=(kc == KC - 1),
                        )
                    nc.scalar.activation(
                        q_sb[:, y0 : y0 + 4, :].rearrange("c r w -> c (r w)"),
                        qk_ps[0:CQK, :],
                        AF.Identity,
                        bias=bq_sb,
                    )
                    nc.scalar.activation(
                        k_sb[:, y0 : y0 + 4, :].rearrange("c r w -> c (r w)"),
                        qk_ps[CQK:128, :],
                        AF.Identity,
                        bias=bk_sb,
                    )

                    # v pixel-major: per row y, out[x, c512] with
                    # lhsT = x16 chunk [cin, x], rhs = wvT [cin, c512]
                    vt16 = v16p.tile([128, 4, C], F16, tag="v16")
                    for j in range(4):
                        v_ps = p1ps.tile([128, C], F32, tag="vps", bufs=4)
                        for kc in range(KC):
                            nc.tensor.matmul(
                                v_ps,
                                x16[:, kc, j * 128 : (j + 1) * 128],
                                wvT[:, kc, :],
                                start=(kc == 0),
                                stop=(kc == KC - 1),
                            )
                        nc.vector.tensor_tensor(
                            vt16[:, j, :], v_ps, bv_full, ALU.add
                        )
                    nc.scalar.dma_start(
                        out=vT_scr[y0 : y0 + 4, :, :].rearrange(
                            "y x c -> x y c"
                        ),
                        in_=vt16,
                    )

            # ---------------- P2: softmax ----------------------------------
            trash = p12.enter_context(tc.tile_pool(name="trash", bufs=4))
            emsk = p12.enter_context(tc.tile_pool(name="emsk", bufs=4))
            ring = p12.enter_context(tc.tile_pool(name="ring", bufs=2))

            with tc.tile_pool(name="p2ps", bufs=1, space="PSUM") as p2ps:
                # ---- sum pass (fp32 exp; no max subtraction) -------------
                for y0 in range(0, H, 4):
                    e_ps = p2ps.tile([128, 4, 128], F32, tag="e_ps", bufs=6)
                    for j in range(4):
                        nc.tensor.matmul(
                            e_ps[:, j, :],
                            q_sb[:, y0 + j, :],
                            k_sb[:, y0 + j, :],
                            start=True,
                            stop=True,
                        )
                    tr = trash.tile([128, 4, 128], F32, tag="trash")
                    nc.scalar.activation(
                        tr.rearrange("p a b -> p (a b)"),
                        e_ps.rearrange("p a b -> p (a b)"),
                        AF.Exp,
                    )
                    nc.vector.reduce_sum(s1[:, y0 : y0 + 4], tr, axis=AX.X)
                for x0 in range(0, W, 4):
                    e_ps = p2ps.tile([128, 4, 128], F32, tag="e_ps", bufs=6)
                    for j in range(4):
                        nc.tensor.matmul(
                            e_ps[:, j, :],
                            q_sb[:, :, x0 + j],
                            k_sb[:, :, x0 + j],
                            start=True,
                            stop=True,
                        )
                    em = emsk.tile([128, 4, 128], F32, tag="emsk")
                    nc.vector.tensor_tensor(
                        em.rearrange("p a b -> p (a b)"),
                        e_ps.rearrange("p a b -> p (a b)"),
                        diag_neg4.rearrange("p a b -> p (a b)"),
                        ALU.add,
                    )
                    tr = trash.tile([128, 4, 128], F32, tag="trash")
                    nc.scalar.activation(
                        tr.rearrange("p a b -> p (a b)"),
                        em.rearrange("p a b -> p (a b)"),
                        AF.Exp,
                    )
                    nc.vector.reduce_sum(s2[:, x0 : x0 + 4], tr, axis=AX.X)

                # ---- nb[y,x] = -(ln(Z) - ln(gamma)); ln via exponent
                # extraction so any fp32 Z is in the ACT Ln table range ----
                zt_ps = p2ps.tile([128, 128], F32, tag="zt", bufs=1)
                nc.tensor.transpose(zt_ps, s1, ident32)
                z_yx = stats.tile([128, W], F32)
                nc.vector.tensor_tensor(z_yx, zt_ps, s2, ALU.add)
                z_i = z_yx[...].bitcast(mybir.dt.int32)
                e_i32 = stats.tile([128, W], mybir.dt.int32)
                nc.vector.tensor_scalar(
                    out=e_i32,
                    in0=z_i,
                    scalar1=23,
                    scalar2=None,
                    op0=ALU.logical_shift_right,
                )
                ef = stats.tile([128, W], F32)
                nc.vector.tensor_scalar(
                    out=ef,
                    in0=e_i32,
                    scalar1=127,
                    scalar2=None,
                    op0=ALU.subtract,
                )
                mant = stats.tile([128, W], F32)
                nc.vector.tensor_scalar(
                    out=mant[...].bitcast(mybir.dt.int32),
                    in0=z_i,
                    scalar1=0x007FFFFF,
                    scalar2=0x3F800000,
                    op0=ALU.bitwise_and,
                    op1=ALU.bitwise_or,
                )
                lnm = stats.tile([128, W], F32)
                nc.scalar.activation(lnm, mant, AF.Ln)
                lnz = stats.tile([128, W], F32)
                nc.vector.scalar_tensor_tensor(
                    out=lnz,
                    in0=ef,
                    scalar=float(np.log(2.0)),
                    in1=lnm,
                    op0=ALU.mult,
                    op1=ALU.add,
                )
                nc.vector.tensor_scalar(
                    out=nb_yx,
                    in0=lnz,
                    scalar1=lng,
                    scalar2=-1.0,
                    op0=ALU.subtract,
                    op1=ALU.mult,
                )
                # transpose nb to [x, y] for the row a-pass bias
                nbt_ps = p2ps.tile([128, 128], F32, tag="zt", bufs=1)
                nc.tensor.transpose(nbt_ps, nb_yx, ident32)
                nc.vector.tensor_copy(nb_xy, nbt_ps)

                # ---- a passes: a = exp(e + nb) via per-partition ACT bias,
                # through ring buffers + batched xbar transpose ------------
                for ycb in range(H // 16):
                    rt = ring.tile([128, 16, 128], F16, tag="ring")
                    for j4 in range(4):
                        y0 = ycb * 16 + j4 * 4
                        e_ps = p2ps.tile(
                            [128, 4, 128], F32, tag="e_ps", bufs=6
                        )
                        for j in range(4):
                            nc.tensor.matmul(
                                e_ps[:, j, :],
                                q_sb[:, y0 + j, :],
                                k_sb[:, y0 + j, :],
                                start=True,
                                stop=True,
                            )
                        for j in range(4):
                            nc.scalar.activation(
                                rt[:, j4 * 4 + j, :],
                                e_ps[:, j, :],
                                AF.Exp,
                                bias=nb_xy[:, y0 + j : y0 + j + 1],
                            )
                    nc.sync.dma_start(
                        out=a_rowT_t[:, ycb * 16 : (ycb + 1) * 16, :],
                        in_=rt.rearrange("p a b -> p (a b)"),
                        transpose=True,
                    )
                for xcb in range(W // 16):
                    rt = ring.tile([128, 16, 128], F16, tag="ring")
                    for j4 in range(4):
                        x0 = xcb * 16 + j4 * 4
                        e_ps = p2ps.tile(
                            [128, 4, 128], F32, tag="e_ps", bufs=6
                        )
                        for j in range(4):
                            nc.tensor.matmul(
                                e_ps[:, j, :],
                                q_sb[:, :, x0 + j],
                                k_sb[:, :, x0 + j],
                                start=True,
                                stop=True,
                            )
                        for j in range(4):
                            nc.scalar.activation(
                                rt[:, j4 * 4 + j, :],
                                e_ps[:, j, :],
                                AF.Exp,
                                bias=nb_yx[:, x0 + j : x0 + j + 1],
                            )
                    nc.sync.dma_start(
                        out=a_colT_t[:, xcb * 16 : (xcb + 1) * 16, :],
                        in_=rt.rearrange("p a b -> p (a b)"),
                        transpose=True,
                    )
                # self pixel: zero the (g == y_out) diagonal across all x
                nc.gpsimd.affine_select(
                    out=a_colT_t,
                    in_=a_colT_t,
                    compare_op=ALU.not_equal,
                    fill=0.0,
                    base=0,
                    pattern=[[0, W], [-1, 128]],
                    channel_multiplier=1,
                )

        # ==================================================================
        # P3: attention application, channel-major output, no transposes.
        # Two 256-channel halves; col pass writes acc, row pass adds
        # acc + x and stores.
        # ==================================================================
        with ExitStack() as p3:
            accp = p3.enter_context(tc.tile_pool(name="accp", bufs=1))
            vrow = p3.enter_context(tc.tile_pool(name="vrow", bufs=3))
            vcol = p3.enter_context(tc.tile_pool(name="vcol", bufs=3))
            xres = p3.enter_context(tc.tile_pool(name="xres", bufs=3))
            outp = p3.enter_context(tc.tile_pool(name="outp", bufs=3))
            tmpp = p3.enter_context(tc.tile_pool(name="tmpp", bufs=3))

            for ch in range(2):  # channel half: c in [ch*256, (ch+1)*256)
                c0 = ch * 256
                acc = accp.tile([128, 2, W, H], F16, tag="acc")  # (c, o, x, y)

                # ---- col pass: acc[c, o, y, x] = col attention ----------
                with tc.tile_pool(name=f"colps{ch}", bufs=1, space="PSUM") as cps:
                    for xg in range(W // 4):
                        x0 = 4 * xg
                        vc = vcol.tile([128, 4, 256], F16, tag="vc")
                        nc.sync.dma_start(
                            out=vc,
                            in_=vT_scr[:, x0 : x0 + 4, c0 : c0 + 256],
                        )
                        for xi in range(4):
                            x = x0 + xi
                            cc_ps = cps.tile(
                                [128, 2, 128], F32, tag="ccps", bufs=4
                            )
                            for o in range(2):
                                nc.tensor.matmul(
                                    cc_ps[:, o, :],
                                    vc[:, xi, o * 128 : (o + 1) * 128],
                                    a_colT_t[:, x, :],
                                    start=True,
                                    stop=True,
                                )
                            if xi % 2 == 0:
                                nc.vector.tensor_copy(acc[:, :, x, :], cc_ps)
                            else:
                                nc.scalar.copy(acc[:, :, x, :], cc_ps)

                # ---- row pass: out = row attention + acc + x ------------
                with tc.tile_pool(name=f"rowps{ch}", bufs=1, space="PSUM") as rps:
                    for yg in range(H // 4):
                        y0 = 4 * yg
                        vr = vrow.tile([128, 4, 256], F16, tag="vr")
                        nc.scalar.dma_start(
                            out=vr,
                            in_=vT_scr[y0 : y0 + 4, :, c0 : c0 + 256].rearrange(
                                "y x c -> x y c"
                            ),
                        )
                        for o in range(2):
                            co = c0 + o * 128
                            xr = xres.tile([128, 4, 128], F16, tag="xr")
                            nc.scalar.dma_start(
                                out=xr.rearrange("c r w -> c (r w)"),
                                in_=x16_scr[
                                    co : co + 128, y0 : y0 + 4, :
                                ].rearrange("c r w -> c (r w)"),
                            )
                            or_ps = rps.tile(
                                [128, 4, 128], F32, tag="orps", bufs=4
                            )
                            for j in range(4):
                                nc.tensor.matmul(
                                    or_ps[:, j, :],
                                    vr[:, j, o * 128 : (o + 1) * 128],
                                    a_rowT_t[:, y0 + j, :],
                                    start=(j == 0),
                                    stop=False,
                                )
                            for j in range(4):
                                nc.tensor.matmul(
                                    or_ps[:, j, :],
                                    ident16,
                                    acc[:, o, :, y0 + j],
                                    start=False,
                                    stop=False,
                                )
                            nc.tensor.matmul(
                                or_ps.rearrange("c r w -> c (r w)"),
                                ident16,
                                xr.rearrange("c r w -> c (r w)"),
                                start=False,
                                stop=True,
                            )
                            ot = outp.tile([128, 4, 128], F32, tag="ot")
                            if o % 2 == 0:
                                nc.vector.tensor_copy(
                                    ot.rearrange("c r w -> c (r w)"),
                                    or_ps.rearrange("c r w -> c (r w)"),
                                )
                            else:
                                nc.scalar.copy(
                                    ot.rearrange("c r w -> c (r w)"),
                                    or_ps.rearrange("c r w -> c (r w)"),
                                )
                            nc.sync.dma_start(
                                out=out_d[
                                    co : co + 128, y0 : y0 + 4, :
                                ].rearrange("c r w -> c (r w)"),
                                in_=ot.rearrange("p a b -> p (a b)"),
                            )

    nc.finalize()
    return nc


_NC_CACHE = {}


def _get_nc():
    if "nc" not in _NC_CACHE:
        _NC_CACHE["nc"] = build()
    return _NC_CACHE["nc"]


def kernel(**inputs) -> np.ndarray:
    x = np.ascontiguousarray(np.asarray(inputs["x"], dtype=np.float32))
    n = x.shape[0]
    assert x.shape == (n, C, H, W)
    shared = {
        name: np.ascontiguousarray(np.asarray(inputs[name], dtype=np.float32))
        for name in ("Wq", "bq", "Wk", "bk", "Wv", "bv", "gamma")
    }
    nc = _get_nc()
    in_maps = [{"x": x[i], **shared} for i in range(n)]
    res = run_bass_kernel_spmd(nc, in_maps, core_ids=list(range(n)))
    return np.stack([res.results[i]["out"] for i in range(n)], axis=0)


if __name__ == "__main__":
    rng = np.random.default_rng(0)
    demo = {
        "x": rng.standard_normal((N_CORES, C, H, W), dtype=np.float32),
        "Wq": rng.standard_normal((CQK, C), dtype=np.float32) / np.sqrt(C),
        "bq": np.zeros(CQK, np.float32),
        "Wk": rng.standard_normal((CQK, C), dtype=np.float32) / np.sqrt(C),
        "bk": np.zeros(CQK, np.float32),
        "Wv": rng.standard_normal((C, C), dtype=np.float32) / np.sqrt(C),
        "bv": np.zeros(C, np.float32),
        "gamma": np.ones(1, np.float32),
    }
    out = kernel(**demo)
    print("out", out.shape, out.dtype, np.abs(out).mean())
